# revision 3
# baseline (speedup 1.0000x reference)
"""Trainium2 (Bass) kernel for the hypervector GNN encoder.

Pipeline (matches the reference nn.Module):
  1. pagerank (10 power iters) -> argsort -> inverse permutation
  2. to_undirected dedup of the 16k edges
  3. out[d] = sum over unique edges e of X[inv[lo_e], d] * X[inv[hi_e], d]

Step 3 touches 2 * 16k * 40KB = 1.28 GB of the 400 MB node table and
dominates end to end (memory-bound). It runs on 8 NeuronCores, sharded over
the hypervector dim D: each core holds a [N, 1280] f32 slice of the table in
HBM, gathers both endpoint rows per edge with SWDGE dma_gather, multiplies
elementwise on the vector engine, and accumulates into a [128, 4, 1280]
partial-sum tile. The host reduces the partials (f64) and concatenates the 8
D-slices.

Steps 1-2 are tiny (16k-edge SpMV iterations + two argsorts) and numerically
delicate: the argsort permutation feeds the gather, so pagerank must be
bit-exact vs the reference. They are computed once on host CPU with the exact
op sequence of the reference (jax CPU backend).
"""

import numpy as np

N_NODES = 10000
HV_DIM = 10000
ALPHA = 0.85
PR_ITERS = 10

NROWS = N_NODES + 1  # row N_NODES is all-zeros (padding target for dup edges)
EW = 1280  # per-core D-slice width (8 * 1280 = 10240 >= 10000)
BLK = 512  # edges per gather block
G = BLK // 128  # accumulator groups per block
N_CORES = 8


def _host_indices(edge_index: np.ndarray) -> tuple[np.ndarray, np.ndarray]:
    """Pagerank -> rank permutation -> deduped undirected edge endpoint rows.

    Bit-exact replica of the reference ops on the CPU jax backend (the rank
    permutation is sensitive to the last ulp of the pagerank vector; CPU jax
    is the only backend the reference itself can run on).
    """
    import jax
    import jax.numpy as jnp
    from jax import lax

    N = N_NODES
    cpu = jax.devices("cpu")[0]

    def _impl(edge_index):
        row, col = edge_index[0], edge_index[1]
        dtype = jnp.float32
        counts = jax.ops.segment_sum(
            jnp.ones_like(col, dtype=dtype), col, num_segments=N
        )
        vals = ALPHA / counts[col]
        p = jnp.asarray((1.0 - ALPHA) / N, dtype=dtype)
        v0 = jnp.full((N,), 1.0 / N, dtype=dtype)

        def step(v, _):
            v = jax.ops.segment_sum(vals * v[col], row, num_segments=N) + p
            return v, None

        v, _ = lax.scan(step, v0, None, length=PR_ITERS)

        perm = jnp.argsort(v)
        inv = (
            jnp.zeros((N,), dtype=jnp.int32)
            .at[perm]
            .set(jnp.arange(N, dtype=jnp.int32))
        )

        lo = jnp.minimum(row, col)
        hi = jnp.maximum(row, col)
        ekey = lo * jnp.int32(N) + hi
        order = jnp.argsort(ekey)
        skey = ekey[order]
        first = jnp.concatenate([jnp.ones((1,), dtype=bool), skey[1:] != skey[:-1]])
        slo = lo[order]
        shi = hi[order]
        return inv[slo], inv[shi], first

    with jax.default_device(cpu):
        ei = jax.device_put(np.asarray(edge_index), cpu)
        ia, ib, first = jax.jit(_impl, backend="cpu")(ei)
        ia, ib, first = np.asarray(ia), np.asarray(ib), np.asarray(first)
    return ia[first], ib[first]


_NC_CACHE: dict = {}


def _build_nc(nb: int, bufs: int = 2):
    """Per-core bass program; nb = number of BLK-edge gather blocks."""
    if (nb, bufs) in _NC_CACHE:
        return _NC_CACHE[(nb, bufs)]
    from contextlib import ExitStack

    import concourse.bacc as bacc
    import concourse.bass as bass
    import concourse.mybir as mybir
    from concourse import library_config
    from concourse._compat import get_trn_type

    DT = mybir.dt.float32
    S = nb * BLK // 16  # idx columns (16-way wrapped)
    C = BLK // 16  # idx columns per block

    nc = bacc.Bacc(get_trn_type() or "TRN2")
    table = nc.dram_tensor("table", [NROWS, EW], DT, kind="ExternalInput")
    idx_a = nc.dram_tensor("idx_a", [128, S], mybir.dt.int16, kind="ExternalInput")
    idx_b = nc.dram_tensor("idx_b", [128, S], mybir.dt.int16, kind="ExternalInput")
    out = nc.dram_tensor("out", [128, G, EW], DT, kind="ExternalOutput")

    with (
        ExitStack() as stack,
        nc.sbuf_tensor("idxa_sb", [128, S], mybir.dt.int16) as idxa_sb,
        nc.sbuf_tensor("idxb_sb", [128, S], mybir.dt.int16) as idxb_sb,
        nc.sbuf_tensor("abuf", [128, bufs, G, EW], DT) as abuf,
        nc.sbuf_tensor("bbuf", [128, bufs, G, EW], DT) as bbuf,
        nc.sbuf_tensor("prod", [128, G, EW], DT) as prod,
        nc.sbuf_tensor("acc", [128, G, EW], DT) as acc,
        nc.semaphore("io_sem") as io_sem,
        nc.semaphore("o_sem") as o_sem,
        nc.semaphore("vop_sem") as vop_sem,
        nc.Block() as block,
    ):
        # one sem per (stream, slot): at most one gather in flight per sem,
        # so intermediate waits are unambiguous
        ga = [stack.enter_context(nc.semaphore(f"ga{s}")) for s in range(bufs)]  # noqa: ANT232
        gb = [stack.enter_context(nc.semaphore(f"gb{s}")) for s in range(bufs)]  # noqa: ANT232

        @block.gpsimd
        def _(gp: bass.BassGpSimd):
            gp.load_library(library_config.mlp)
            gp.dma_start(idxa_sb[:, :], idx_a[:, :]).then_inc(io_sem, 16)
            gp.dma_start(idxb_sb[:, :], idx_b[:, :]).then_inc(io_sem, 16)
            gp.wait_ge(io_sem, 32)
            for i in range(nb):
                s = i % bufs
                if i >= bufs:
                    # slot s last used by block i-bufs; DVE finishes that block
                    # at vop_sem == 2*(i-bufs)+1 (block 0 is 1 op, others 2)
                    gp.wait_ge(vop_sem, 2 * (i - bufs) + 1)
                cols = bass.ds(i * C, C)
                gp.dma_gather(
                    abuf[:, s], table[:, :], idxa_sb[:, cols], BLK, BLK, EW
                ).then_inc(ga[s], 16)
                gp.dma_gather(
                    bbuf[:, s], table[:, :], idxb_sb[:, cols], BLK, BLK, EW
                ).then_inc(gb[s], 16)

        @block.vector
        def _(v: bass.BassEngine):
            nops = 0  # DVE ops completed so far (chains same-engine RAW/WAR)
            for i in range(nb):
                s = i % bufs
                k = i // bufs + 1
                v.wait_ge(ga[s], 16 * k)
                v.wait_ge(gb[s], 16 * k)
                if i == 0:
                    v.tensor_mul(acc[:, :, :], abuf[:, s], bbuf[:, s]).then_inc(
                        vop_sem, 1
                    )
                    nops = 1
                else:
                    v.wait_ge(vop_sem, nops)
                    v.tensor_mul(prod[:, :, :], abuf[:, s], bbuf[:, s]).then_inc(
                        vop_sem, 1
                    )
                    nops += 1
                    v.wait_ge(vop_sem, nops)
                    v.tensor_add(acc[:, :, :], acc[:, :, :], prod[:, :, :]).then_inc(
                        vop_sem, 1
                    )
                    nops += 1

        @block.sync
        def _(sy: bass.BassEngine):
            sy.wait_ge(vop_sem, 2 * nb - 1)
            sy.dma_start(out[:, :, :], acc[:, :, :]).then_inc(o_sem, 16)
            sy.wait_ge(o_sem, 16)

    nc.compile()
    _NC_CACHE[(nb, bufs)] = nc
    return nc


def _wrap_idx(idx: np.ndarray) -> np.ndarray:
    """[Epad] -> [128, Epad//16] int16, 16-way wrapped, replicated x8."""
    w = idx.astype(np.int16).reshape(-1, 16).T  # [16, S]
    return np.tile(w, (8, 1))


def kernel(edge_index: np.ndarray, node_ids: np.ndarray) -> np.ndarray:
    edge_index = np.asarray(edge_index)
    node_ids = np.asarray(node_ids, dtype=np.float32)
    N, D = node_ids.shape
    assert (N, D) == (N_NODES, HV_DIM)

    a, b = _host_indices(edge_index)

    eu = len(a)
    nb = -(-eu // BLK)
    epad = nb * BLK
    a_p = np.full(epad, N_NODES, dtype=np.int16)
    b_p = np.full(epad, N_NODES, dtype=np.int16)
    a_p[:eu] = a
    b_p[:eu] = b
    ia = _wrap_idx(a_p)
    ib = _wrap_idx(b_p)

    in_maps = []
    for c in range(N_CORES):
        tbl = np.zeros((NROWS, EW), dtype=np.float32)
        lo = c * EW
        hi = min(lo + EW, D)
        tbl[:N_NODES, : hi - lo] = node_ids[:, lo:hi]
        in_maps.append({"table": tbl, "idx_a": ia, "idx_b": ib})

    nc = _build_nc(nb)

    from concourse.bass_utils import run_bass_kernel_spmd

    res = run_bass_kernel_spmd(nc, in_maps, core_ids=list(range(N_CORES)))

    out = np.empty(D, dtype=np.float32)
    for c in range(N_CORES):
        accum = res.results[c]["out"].astype(np.float64).sum(axis=(0, 1))  # [EW]
        lo = c * EW
        hi = min(lo + EW, D)
        out[lo:hi] = accum[: hi - lo].astype(np.float32)
    return out


# revision 4
# speedup vs baseline: 1.3753x; 1.3753x over previous
"""Trainium2 (Bass) kernel for the hypervector GNN encoder.

Pipeline (matches the reference nn.Module):
  1. pagerank (10 power iters) -> argsort -> inverse permutation
  2. to_undirected dedup of the 16k edges
  3. out[d] = sum over unique edges e of X[inv[lo_e], d] * X[inv[hi_e], d]

Step 3 touches 2 * 16k * 40KB = 1.28 GB of the 400 MB node table and
dominates end to end (memory-bound). It runs on 8 NeuronCores, sharded over
the hypervector dim D: each core holds a [N, 1280] f32 slice of the table in
HBM, gathers both endpoint rows per edge with SWDGE dma_gather, multiplies
elementwise on the vector engine, and accumulates into a [128, 4, 1280]
partial-sum tile. The host reduces the partials (f64) and concatenates the 8
D-slices.

Steps 1-2 are tiny (16k-edge SpMV iterations + two argsorts) and numerically
delicate: the argsort permutation feeds the gather, so pagerank must be
bit-exact vs the reference. They are computed once on host CPU with the exact
op sequence of the reference (jax CPU backend).
"""

import numpy as np

N_NODES = 10000
HV_DIM = 10000
ALPHA = 0.85
PR_ITERS = 10

NROWS = N_NODES + 1  # row N_NODES is all-zeros (padding target for dup edges)
EW = 1280  # per-core D-slice width (8 * 1280 = 10240 >= 10000)
BLK = 512  # edges per gather block
G = BLK // 128  # accumulator groups per block
N_CORES = 8


def _host_indices(edge_index: np.ndarray) -> tuple[np.ndarray, np.ndarray]:
    """Pagerank -> rank permutation -> deduped undirected edge endpoint rows.

    Bit-exact replica of the reference ops on the CPU jax backend (the rank
    permutation is sensitive to the last ulp of the pagerank vector; CPU jax
    is the only backend the reference itself can run on).
    """
    import jax
    import jax.numpy as jnp
    from jax import lax

    N = N_NODES
    cpu = jax.devices("cpu")[0]

    def _impl(edge_index):
        row, col = edge_index[0], edge_index[1]
        dtype = jnp.float32
        counts = jax.ops.segment_sum(
            jnp.ones_like(col, dtype=dtype), col, num_segments=N
        )
        vals = ALPHA / counts[col]
        p = jnp.asarray((1.0 - ALPHA) / N, dtype=dtype)
        v0 = jnp.full((N,), 1.0 / N, dtype=dtype)

        def step(v, _):
            v = jax.ops.segment_sum(vals * v[col], row, num_segments=N) + p
            return v, None

        v, _ = lax.scan(step, v0, None, length=PR_ITERS)

        perm = jnp.argsort(v)
        inv = (
            jnp.zeros((N,), dtype=jnp.int32)
            .at[perm]
            .set(jnp.arange(N, dtype=jnp.int32))
        )

        lo = jnp.minimum(row, col)
        hi = jnp.maximum(row, col)
        ekey = lo * jnp.int32(N) + hi
        order = jnp.argsort(ekey)
        skey = ekey[order]
        first = jnp.concatenate([jnp.ones((1,), dtype=bool), skey[1:] != skey[:-1]])
        slo = lo[order]
        shi = hi[order]
        return inv[slo], inv[shi], first

    with jax.default_device(cpu):
        ei = jax.device_put(np.asarray(edge_index), cpu)
        try:
            fn = jax.jit(_impl, backend="cpu")
        except TypeError:  # newer jax without the backend= param
            fn = jax.jit(_impl)
        ia, ib, first = fn(ei)
        ia, ib, first = np.asarray(ia), np.asarray(ib), np.asarray(first)
    return ia[first], ib[first]


_NC_CACHE: dict = {}


def _build_nc(nb: int, bufs: int = 2):
    """Per-core bass program; nb = number of BLK-edge gather blocks."""
    if (nb, bufs) in _NC_CACHE:
        return _NC_CACHE[(nb, bufs)]
    from contextlib import ExitStack

    import concourse.bacc as bacc
    import concourse.bass as bass
    import concourse.mybir as mybir
    from concourse import library_config
    from concourse._compat import get_trn_type

    DT = mybir.dt.float32
    S = nb * BLK // 16  # idx columns (16-way wrapped)
    C = BLK // 16  # idx columns per block

    nc = bacc.Bacc(get_trn_type() or "TRN2")
    table = nc.dram_tensor("table", [NROWS, EW], DT, kind="ExternalInput")
    idx_a = nc.dram_tensor("idx_a", [128, S], mybir.dt.int16, kind="ExternalInput")
    idx_b = nc.dram_tensor("idx_b", [128, S], mybir.dt.int16, kind="ExternalInput")
    out = nc.dram_tensor("out", [128, G, EW], DT, kind="ExternalOutput")

    with (
        ExitStack() as stack,
        nc.sbuf_tensor("idxa_sb", [128, S], mybir.dt.int16) as idxa_sb,
        nc.sbuf_tensor("idxb_sb", [128, S], mybir.dt.int16) as idxb_sb,
        nc.sbuf_tensor("abuf", [128, bufs, G, EW], DT) as abuf,
        nc.sbuf_tensor("bbuf", [128, bufs, G, EW], DT) as bbuf,
        nc.sbuf_tensor("prod", [128, G, EW], DT) as prod,
        nc.sbuf_tensor("acc", [128, G, EW], DT) as acc,
        nc.semaphore("io_sem") as io_sem,
        nc.semaphore("o_sem") as o_sem,
        nc.semaphore("vop_sem") as vop_sem,
        nc.Block() as block,
    ):
        # one sem per (stream, slot): at most one gather in flight per sem,
        # so intermediate waits are unambiguous
        ga = [stack.enter_context(nc.semaphore(f"ga{s}")) for s in range(bufs)]  # noqa: ANT232
        gb = [stack.enter_context(nc.semaphore(f"gb{s}")) for s in range(bufs)]  # noqa: ANT232

        @block.gpsimd
        def _(gp: bass.BassGpSimd):
            gp.load_library(library_config.mlp)
            gp.dma_start(idxa_sb[:, :], idx_a[:, :]).then_inc(io_sem, 16)
            gp.dma_start(idxb_sb[:, :], idx_b[:, :]).then_inc(io_sem, 16)
            gp.wait_ge(io_sem, 32)
            for i in range(nb):
                s = i % bufs
                if i >= bufs:
                    # slot s last used by block i-bufs; DVE finishes that block
                    # at vop_sem == 2*(i-bufs)+1 (block 0 is 1 op, others 2)
                    gp.wait_ge(vop_sem, 2 * (i - bufs) + 1)
                cols = bass.ds(i * C, C)
                gp.dma_gather(
                    abuf[:, s], table[:, :], idxa_sb[:, cols], BLK, BLK, EW
                ).then_inc(ga[s], 16)
                gp.dma_gather(
                    bbuf[:, s], table[:, :], idxb_sb[:, cols], BLK, BLK, EW
                ).then_inc(gb[s], 16)

        @block.vector
        def _(v: bass.BassEngine):
            nops = 0  # DVE ops completed so far (chains same-engine RAW/WAR)
            for i in range(nb):
                s = i % bufs
                k = i // bufs + 1
                v.wait_ge(ga[s], 16 * k)
                v.wait_ge(gb[s], 16 * k)
                if i == 0:
                    v.tensor_mul(acc[:, :, :], abuf[:, s], bbuf[:, s]).then_inc(
                        vop_sem, 1
                    )
                    nops = 1
                else:
                    v.wait_ge(vop_sem, nops)
                    v.tensor_mul(prod[:, :, :], abuf[:, s], bbuf[:, s]).then_inc(
                        vop_sem, 1
                    )
                    nops += 1
                    v.wait_ge(vop_sem, nops)
                    v.tensor_add(acc[:, :, :], acc[:, :, :], prod[:, :, :]).then_inc(
                        vop_sem, 1
                    )
                    nops += 1

        @block.sync
        def _(sy: bass.BassEngine):
            sy.wait_ge(vop_sem, 2 * nb - 1)
            sy.dma_start(out[:, :, :], acc[:, :, :]).then_inc(o_sem, 16)
            sy.wait_ge(o_sem, 16)

    nc.compile()
    _NC_CACHE[(nb, bufs)] = nc
    return nc


def _wrap_idx(idx: np.ndarray) -> np.ndarray:
    """[Epad] -> [128, Epad//16] int16, 16-way wrapped, replicated x8."""
    w = idx.astype(np.int16).reshape(-1, 16).T  # [16, S]
    return np.tile(w, (8, 1))


def kernel(edge_index: np.ndarray, node_ids: np.ndarray) -> np.ndarray:
    edge_index = np.asarray(edge_index)
    node_ids = np.asarray(node_ids, dtype=np.float32)
    N, D = node_ids.shape
    assert (N, D) == (N_NODES, HV_DIM)

    a, b = _host_indices(edge_index)

    eu = len(a)
    nb = -(-eu // BLK)
    epad = nb * BLK
    a_p = np.full(epad, N_NODES, dtype=np.int16)
    b_p = np.full(epad, N_NODES, dtype=np.int16)
    a_p[:eu] = a
    b_p[:eu] = b
    ia = _wrap_idx(a_p)
    ib = _wrap_idx(b_p)

    in_maps = []
    for c in range(N_CORES):
        tbl = np.zeros((NROWS, EW), dtype=np.float32)
        lo = c * EW
        hi = min(lo + EW, D)
        tbl[:N_NODES, : hi - lo] = node_ids[:, lo:hi]
        in_maps.append({"table": tbl, "idx_a": ia, "idx_b": ib})

    nc = _build_nc(nb)

    from concourse.bass_utils import run_bass_kernel_spmd

    res = run_bass_kernel_spmd(nc, in_maps, core_ids=list(range(N_CORES)))

    out = np.empty(D, dtype=np.float32)
    for c in range(N_CORES):
        accum = res.results[c]["out"].astype(np.float64).sum(axis=(0, 1))  # [EW]
        lo = c * EW
        hi = min(lo + EW, D)
        out[lo:hi] = accum[: hi - lo].astype(np.float32)
    return out
